# revision 1
# baseline (speedup 1.0000x reference)
"""GAT (2-layer, PyG-style) Trainium2 Bass kernel, 8-core SPMD.

Sharding: destinations are range-sharded across 8 cores (6250 nodes each).
Each core:
  - computes the full node table h = x @ [W1 | W1@Asrc | W1@Adst] (replicated),
    writes gather tables to HBM,
  - gathers per-edge rows (by src) and per-edge dst scores (by local dst) with
    SWDGE dma_gather,
  - computes edge scores  e = leakyrelu(as[src]+ad[dst]),  w = exp(e)
    (max-subtraction dropped: it cancels exactly in the softmax ratio),
  - aggregates  out[d] = (sum_e S01[e,d] * w_e * h[src_e]) / (sum_e S01[e,d] * w_e)
    via PSUM-accumulated TensorE matmuls against host-built 0/1 scatter blocks,
  - applies bias+ELU, computes layer-2 node rows, AllGathers them (1.6MB/rank),
  - repeats the gather/aggregate phase for layer 2 and writes its output shard.

All indices / scatter blocks / paddings are host-precomputed per core and fed
as per-core input tensors, so one SPMD NEFF serves all 8 cores.
"""

import math
from dataclasses import dataclass, field

import numpy as np
import ml_dtypes

BF16 = ml_dtypes.bfloat16

P = 128  # partitions / tile edge

# HW bisection: "A"=node tables only, "B"=+layer1 agg, "C"=+allgather, "D"=full
BUILD_STAGE = "D"
EN_GATHERS = "hsd"  # which gather kinds to emit in B0 debugging


@dataclass
class Cfg:
    n_nodes: int = 50000
    n_edges: int = 800000  # before self loops
    f_in: int = 128
    heads: int = 8
    hid: int = 32
    n_cores: int = 8
    group_tiles: int = 2  # dst tiles per gather group
    split: int = 32768  # int16 index split point
    neg_slope: float = 0.2

    @property
    def shard(self):
        return self.n_nodes // self.n_cores

    @property
    def hc(self):
        return self.heads * self.hid  # 256

    @property
    def n_tiles(self):
        return math.ceil(self.shard / P)  # dst tiles per core

    @property
    def nt1(self):
        return math.ceil(self.n_nodes / P)  # node-table tiles

    @property
    def n_pad(self):
        return self.nt1 * P

    @property
    def shard_pad(self):
        return self.n_tiles * P


# ------------------------------------------------------------ host preprocess


@dataclass
class Plan:
    """Structure shared by all cores (uniform) + per-core tensor data."""

    CA: list = field(default_factory=list)  # A-chunks per tile (maxed over cores)
    CB: list = field(default_factory=list)
    groups: list = field(default_factory=list)  # per group: list of tile ids
    g_nA: list = field(default_factory=list)
    g_nB: list = field(default_factory=list)
    g_chunk0: list = field(default_factory=list)
    k_tot: int = 0
    data: list = field(default_factory=list)  # per-core input arrays


def _wrap16(idx: np.ndarray) -> np.ndarray:
    """[n] -> [128, n/16] int16 gather-index layout (16-wrapped, x8 replicated)."""
    n = idx.shape[0]
    assert n % 16 == 0
    a = idx.astype(np.int16).reshape(n // 16, 16).T  # [16, n/16]
    return np.tile(a, (8, 1)).copy()


def preprocess(edge_index: np.ndarray, cfg: Cfg) -> Plan:
    N = cfg.n_nodes
    loop = np.arange(N, dtype=np.int64)
    src = np.concatenate([edge_index[0].astype(np.int64), loop])
    dst = np.concatenate([edge_index[1].astype(np.int64), loop])

    plan = Plan()
    ncores = cfg.n_cores
    shard = cfg.shard
    cdiv = lambda a, b: -(-a // b)

    per_core = []
    for c in range(ncores):
        m = (dst >= c * shard) & (dst < (c + 1) * shard)
        s_c, d_c = src[m], dst[m] - c * shard
        order = np.argsort(d_c, kind="stable")
        s_c, d_c = s_c[order], d_c[order]
        tiles = []
        for t in range(cfg.n_tiles):
            tm = (d_c >= t * P) & (d_c < (t + 1) * P)
            s_t, d_t = s_c[tm], d_c[tm] - t * P
            a = s_t < cfg.split
            tiles.append((s_t[a], d_t[a], s_t[~a], d_t[~a]))
        per_core.append(tiles)

    for t in range(cfg.n_tiles):
        plan.CA.append(max(cdiv(len(per_core[c][t][0]), P) for c in range(ncores)))
        plan.CB.append(max(cdiv(len(per_core[c][t][2]), P) for c in range(ncores)))

    for g0 in range(0, cfg.n_tiles, cfg.group_tiles):
        plan.groups.append(list(range(g0, min(g0 + cfg.group_tiles, cfg.n_tiles))))
    k = 0
    for g in plan.groups:
        plan.g_chunk0.append(k)
        plan.g_nA.append(sum(plan.CA[t] for t in g))
        plan.g_nB.append(sum(plan.CB[t] for t in g))
        k += plan.g_nA[-1] + plan.g_nB[-1]
    plan.k_tot = k

    for c in range(ncores):
        idxA, idxD = [], []
        s01 = np.zeros((plan.k_tot, P, P), dtype=BF16)
        for gi, g in enumerate(plan.groups):
            k0 = plan.g_chunk0[gi]
            nA = plan.g_nA[gi]
            a_off = 0
            b_off = 0
            gA_src, gA_dst, gB_src, gB_dst = [], [], [], []
            for t in g:
                sA, dA, sB, dB = per_core[c][t]
                la, lb = plan.CA[t] * P, plan.CB[t] * P
                sA_p = np.concatenate([sA, np.zeros(la - len(sA), np.int64)])
                sB_p = np.concatenate(
                    [sB - cfg.split, np.zeros(lb - len(sB), np.int64)]
                )
                dA_p = np.concatenate([dA, np.zeros(la - len(dA), np.int64)])
                dB_p = np.concatenate([dB, np.zeros(lb - len(dB), np.int64)])
                gA_src.append(sA_p)
                gB_src.append(sB_p)
                gA_dst.append(t * P + dA_p)
                gB_dst.append(t * P + dB_p)
                if len(sA):
                    jj = np.arange(len(sA))
                    s01[k0 + a_off + jj // P, jj % P, dA] = 1.0
                if len(sB):
                    jj = np.arange(len(sB))
                    s01[k0 + nA + b_off + jj // P, jj % P, dB] = 1.0
                # fake entries so padded dst columns get denom > 0
                width = min(cfg.shard - t * P, P)
                if width < P and plan.CA[t] + plan.CB[t] > 0:
                    kf = (k0 + a_off) if plan.CA[t] > 0 else (k0 + nA + b_off)
                    for d_pad in range(width, P):
                        s01[kf, (d_pad - width) % P, d_pad] = 1.0
                a_off += plan.CA[t]
                b_off += plan.CB[t]
            idxA.append(np.concatenate(gA_src + gB_src))
            idxD.append(np.concatenate(gA_dst + gB_dst))
        cat = lambda xs: (
            np.concatenate([_wrap16(x) for x in xs if len(x)], axis=1)
            if any(len(x) for x in xs)
            else np.zeros((128, 0), np.int16)
        )
        plan.data.append(
            {
                "IDXA": cat(idxA),
                "IDXD": cat(idxD),
                "S01": np.ascontiguousarray(s01.transpose(1, 0, 2)),  # [128,K,128]
            }
        )
    return plan


def prep_weights(inputs: dict, cfg: Cfg):
    W1 = np.asarray(inputs["W1"], np.float32)
    a_s1 = np.asarray(inputs["att_src1"], np.float32)
    a_d1 = np.asarray(inputs["att_dst1"], np.float32)
    W2 = np.asarray(inputs["W2"], np.float32)
    a_s2 = np.asarray(inputs["att_src2"], np.float32)
    a_d2 = np.asarray(inputs["att_dst2"], np.float32)
    H, C = cfg.heads, cfg.hid
    W1r = W1.reshape(cfg.f_in, H, C)
    w1as = np.einsum("fhc,hc->fh", W1r, a_s1)
    w1ad = np.einsum("fhc,hc->fh", W1r, a_d1)
    W1p = np.concatenate([W1, w1as, w1ad], axis=1).astype(BF16)  # [F, HC+16]
    w2as = W2 @ a_s2[0]
    w2ad = W2 @ a_d2[0]
    W2p = np.concatenate([W2, w2as[:, None], w2ad[:, None]], axis=1).astype(BF16)
    b1rep = np.tile(np.asarray(inputs["b1"], np.float32)[None, :], (P, 1))
    b2rep = np.tile(np.asarray(inputs["b2"], np.float32)[None, :], (P, 1))
    return W1p, W2p, b1rep.astype(np.float32), b2rep.astype(np.float32)


# ---------------------------------------------------------------- bass kernel


def build_kernel(cfg: Cfg, plan: Plan):
    from contextlib import ExitStack

    import concourse.bacc as bacc
    import concourse.bass as bass
    import concourse.mybir as mybir
    import concourse.tile as tile

    fp32 = mybir.dt.float32
    bf16 = mybir.dt.bfloat16
    i16 = mybir.dt.int16
    AF = mybir.ActivationFunctionType
    OP = mybir.AluOpType

    HC = cfg.hc  # 256
    HCX = HC + 16
    H = cfg.heads
    HID = cfg.hid
    NTAB = cfg.n_pad
    SH = cfg.shard
    SHP = cfg.shard_pad
    T2W = 128  # layer-2 table row width (256B rows): [h2 | as2 | ad2 | pad]
    NKW2 = HC // P  # 2 chunks for the layer-2 prep matmul

    nc = bacc.Bacc(
        "TRN2",
        num_devices=cfg.n_cores,
        num_swdge_queues=1,
        name="gat8",
    )

    xT = nc.dram_tensor("xT", [P, NTAB], bf16, kind="ExternalInput")
    xTown = nc.dram_tensor("xTown", [P, SHP], bf16, kind="ExternalInput")
    W1p = nc.dram_tensor("W1p", [cfg.f_in, HCX], bf16, kind="ExternalInput")
    W2p = nc.dram_tensor("W2p", [HC, HID + 2], bf16, kind="ExternalInput")
    b1rep = nc.dram_tensor("b1rep", [P, HC], fp32, kind="ExternalInput")
    b2rep = nc.dram_tensor("b2rep", [P, HID], fp32, kind="ExternalInput")
    identity = nc.dram_tensor("identity", [P, P], bf16, kind="ExternalInput")
    d0 = plan.data[0]
    IDXA = nc.dram_tensor("IDXA", list(d0["IDXA"].shape), i16, kind="ExternalInput")
    IDXD = nc.dram_tensor("IDXD", list(d0["IDXD"].shape), i16, kind="ExternalInput")
    S01 = nc.dram_tensor("S01", [P, plan.k_tot, P], bf16, kind="ExternalInput")
    OUT = nc.dram_tensor("out", [SH, HID], fp32, kind="ExternalOutput")

    with tile.TileContext(nc) as tc, ExitStack() as ctx:
        _regs = {}

        def nreg(v):
            if v not in _regs:
                _regs[v] = nc.gpsimd.to_reg(v)
            return _regs[v]

        sb = ctx.enter_context(tc.tile_pool(name="sb", bufs=2))
        sb1 = ctx.enter_context(tc.tile_pool(name="sb1", bufs=1))
        psA = ctx.enter_context(tc.tile_pool(name="psA", bufs=2, space="PSUM"))
        psB = ctx.enter_context(tc.tile_pool(name="psB", bufs=2, space="PSUM"))
        dram = ctx.enter_context(tc.tile_pool(name="dram", bufs=1, space="DRAM"))

        T1x = dram.tile([NTAB, 384], bf16, tag="T1x")  # [h 256|as 8|ad 8|pad]
        T1sloc = dram.tile([SHP, P], bf16, tag="T1sloc")
        T2sh = dram.tile([SH, T2W], bf16, tag="T2sh")
        T2full = dram.tile([cfg.n_nodes, T2W], bf16, tag="T2full")

        # constants
        w1_sb = sb1.tile([cfg.f_in, HCX], bf16, tag="w1")
        nc.sync.dma_start(w1_sb[:], W1p[:])
        w2_sb = sb1.tile([P, NKW2 * (HID + 2)], bf16, tag="w2")
        nc.sync.dma_start(
            w2_sb[:].rearrange("p (a n) -> p a n", a=NKW2),
            W2p[:].rearrange("(a p) n -> p a n", p=P),
        )
        w2_3 = w2_sb[:].rearrange("p (a n) -> p a n", a=NKW2)
        b1_sb = sb1.tile([P, HC], fp32, tag="b1")
        nc.sync.dma_start(b1_sb[:], b1rep[:])
        b2_sb = sb1.tile([P, HID], fp32, tag="b2")
        nc.sync.dma_start(b2_sb[:], b2rep[:])
        id_sb = sb1.tile([P, P], bf16, tag="id")
        nc.sync.dma_start(id_sb[:], identity[:])

        # ---------------- phase 1: node table ----------------
        for i in range(cfg.nt1):
            xt = sb.tile([P, P], bf16, tag="xt")
            nc.sync.dma_start(xt[:], xT[:, i * P : (i + 1) * P])
            pt = psA.tile([P, HCX], fp32, tag="p1")
            nc.tensor.matmul(out=pt[:], lhsT=xt[:], rhs=w1_sb[:], start=True, stop=True)
            stg = sb.tile([P, HCX], bf16, tag="stg1")
            if i % 2 == 0:
                nc.vector.tensor_copy(stg[:], pt[:])
            else:
                nc.scalar.copy(stg[:], pt[:])
            nc.sync.dma_start(T1x[i * P : (i + 1) * P, :HCX], stg[:])

        # phase 1b: own-shard score rows (dst-side gather table)
        for i in range(cfg.n_tiles):
            xo = sb.tile([P, P], bf16, tag="xt")
            nc.sync.dma_start(xo[:], xTown[:, i * P : (i + 1) * P])
            pt = psA.tile([P, 16], fp32, tag="p1")
            nc.tensor.matmul(
                out=pt[:], lhsT=xo[:], rhs=w1_sb[:, HC:HCX], start=True, stop=True
            )
            stg = sb.tile([P, 16], bf16, tag="stg1b")
            nc.scalar.copy(stg[:], pt[:])
            nc.sync.dma_start(T1sloc[i * P : (i + 1) * P, :16], stg[:])

        # ------------- layer-1 per-tile epilogue: bias, ELU, layer-2 rows ----
        def epilogue1(t, o_f):
            y = sb.tile([P, HC], fp32, tag="ep_y")
            nc.vector.tensor_tensor(out=y[:], in0=o_f[:], in1=b1_sb[:], op=OP.add)
            mn = sb.tile([P, HC], fp32, tag="ep_mn")
            nc.vector.tensor_scalar_min(mn[:], y[:], 0.0)
            ex = sb.tile([P, HC], fp32, tag="ep_ex")
            nc.scalar.activation(ex[:], mn[:], AF.Exp)
            nc.vector.tensor_scalar_max(y[:], y[:], 0.0)  # relu, in place
            nc.vector.tensor_tensor(out=y[:], in0=y[:], in1=ex[:], op=OP.add)
            elu_bf = sb.tile([P, HC], bf16, tag="ep_bf")
            nc.vector.tensor_scalar_add(elu_bf[:], y[:], -1.0)
            eluT = sb.tile([P, HC], bf16, tag="ep_eT")
            for j in range(NKW2):
                ptT = psB.tile([P, P], bf16, tag="ptT")
                nc.tensor.transpose(
                    out=ptT[:], in_=elu_bf[:, j * P : (j + 1) * P], identity=id_sb[:]
                )
                nc.scalar.copy(eluT[:, j * P : (j + 1) * P], ptT[:])
            p2 = psB.tile([P, HID + 2], fp32, tag="p2")
            for j in range(NKW2):
                nc.tensor.matmul(
                    out=p2[:],
                    lhsT=eluT[:, j * P : (j + 1) * P],
                    rhs=w2_3[:, j, :],
                    start=(j == 0),
                    stop=(j == NKW2 - 1),
                )
            r2 = sb.tile([P, HID + 2], bf16, tag="r2")
            nc.scalar.copy(r2[:], p2[:])
            rows = min(SH - t * P, P)
            nc.sync.dma_start(T2sh[t * P : t * P + rows, : HID + 2], r2[:rows, :])

        # ---------------- shared gather/aggregate phase ----------------
        def agg_layer(layer):
            elem_h = 384 if layer == 1 else T2W
            nhead = H if layer == 1 else 1
            rhs_w = HC if layer == 1 else HID  # message width

            for gi, g in enumerate(plan.groups):
                nA, nBg = plan.g_nA[gi], plan.g_nB[gi]
                ng = nA + nBg
                k0 = plan.g_chunk0[gi]

                bufh = sb.tile([P, ng * elem_h], bf16, tag="bufh")
                bufh3 = bufh[:].rearrange("p (k e) -> p k e", e=elem_h)
                tabh = T1x if layer == 1 else T2full

                def win_gather(out3, table_ap, idx_tile, c0, n_chunks, elem):
                    for w0 in range(0, n_chunks, 8):
                        wn = min(8, n_chunks - w0)
                        nc.gpsimd.dma_gather(
                            out_ap=out3[:, c0 + w0 : c0 + w0 + wn, :],
                            in_ap=table_ap,
                            idxs_ap=idx_tile[:, (c0 + w0) * 8 : (c0 + w0 + wn) * 8],
                            num_idxs=wn * P,
                            num_idxs_reg=wn * P,
                            elem_size=elem,
                            queue_num=0,
                        )

                ih = sb.tile([P, ng * 8], i16, tag="ia")
                a0 = sum(plan.g_nA[j] + plan.g_nB[j] for j in range(gi)) * 8
                nc.sync.dma_start(ih[:], IDXA[:, a0 : a0 + ng * 8])
                idt = sb.tile([P, ng * 8], i16, tag="idt")
                nc.sync.dma_start(idt[:], IDXD[:, a0 : a0 + ng * 8])
                if nA:
                    win_gather(bufh3, tabh[:, :], ih, 0, nA, elem_h)
                if nBg:
                    win_gather(bufh3, tabh[cfg.split :, :], ih, nA, nBg, elem_h)
                bufd = sb.tile([P, ng * P], bf16, tag="bufd")
                bufd3 = bufd[:].rearrange("p (k e) -> p k e", e=P)
                win_gather(
                    bufd3, (T1sloc if layer == 1 else T2sh)[:, :], idt, 0, ng, P
                )

                # scores: s = as[src] + ad[dst]; w = exp(max(s, 0.2 s))
                nsc = ng * nhead
                s_f = sb.tile([P, nsc], fp32, tag="s_f")
                if layer == 1:
                    as_ap = bufh3[:, :, HC : HC + H]
                    ad_ap = bufd3[:, :, 8 : 8 + H]
                else:
                    as_ap = bufh3[:, :, HID : HID + 1]
                    ad_ap = bufd3[:, :, HID + 1 : HID + 2]
                nc.vector.tensor_tensor(out=s_f[:], in0=as_ap, in1=ad_ap, op=OP.add)
                s_lr = sb.tile([P, nsc], fp32, tag="s_lr")
                nc.scalar.mul(s_lr[:], s_f[:], cfg.neg_slope)
                nc.vector.tensor_tensor(out=s_lr[:], in0=s_lr[:], in1=s_f[:], op=OP.max)
                w_bf = sb.tile([P, nsc], bf16, tag="w_bf")
                nc.scalar.activation(w_bf[:], s_lr[:], AF.Exp)
                # copy w into the padding columns of the gathered rows so the
                # aggregation matmul also produces softmax denominators
                wcol = 272 if layer == 1 else 36
                nc.scalar.copy(bufh3[:, :, wcol : wcol + nhead], w_bf[:])

                # premultiply gathered message rows by w (in place)
                if layer == 1:
                    mw_io = bufh3[:, :, :HC].rearrange("p k (h c) -> p k h c", c=HID)
                    w_b = w_bf[:].rearrange("p (k h) -> p k h", h=H).to_broadcast(
                        [P, ng, H, HID]
                    )
                else:
                    mw_io = bufh3[:, :, :HID]
                    w_b = w_bf[:].to_broadcast([P, ng, HID])
                nc.vector.tensor_tensor(out=mw_io, in0=mw_io, in1=w_b, op=OP.mult)

                s01_sb = sb.tile([P, ng * P], bf16, tag="s01")
                nc.sync.dma_start(s01_sb[:], S01[:, k0 : k0 + ng, :])
                s01_3 = s01_sb[:].rearrange("p (k d) -> p k d", d=P)

                a_off, b_off = 0, 0
                rw = wcol + nhead  # matmul rhs width incl junk + w cols
                for t in g:
                    pt = psA.tile([P, rw], fp32, tag="pagg")
                    chunks = [a_off + j for j in range(plan.CA[t])] + [
                        nA + b_off + j for j in range(plan.CB[t])
                    ]
                    nk = len(chunks)
                    for ci, k in enumerate(chunks):
                        nc.tensor.matmul(
                            out=pt[:],
                            lhsT=s01_3[:, k, :],
                            rhs=bufh3[:, k, :rw],
                            start=(ci == 0),
                            stop=(ci == nk - 1),
                        )
                    a_off += plan.CA[t]
                    b_off += plan.CB[t]

                    den_r = sb.tile([P, nhead], fp32, tag="denr")
                    nc.vector.reciprocal(den_r[:], pt[:, wcol : wcol + nhead])
                    o_f = sb.tile([P, rhs_w], fp32, tag="o_f")
                    if layer == 1:
                        nc.vector.tensor_tensor(
                            out=o_f[:].rearrange("p (h c) -> p h c", c=HID),
                            in0=pt[:, :rhs_w].rearrange("p (h c) -> p h c", c=HID),
                            in1=den_r[:].to_broadcast([P, nhead, HID]),
                            op=OP.mult,
                        )
                        if BUILD_STAGE == "B2":
                            rows = min(SH - t * P, P)
                            nc.sync.dma_start(
                                T2sh[t * P : t * P + rows, 64:96], o_f[:rows, :32]
                            )
                        else:
                            epilogue1(t, o_f)
                    else:
                        nc.vector.tensor_tensor(
                            out=o_f[:],
                            in0=pt[:, :HID],
                            in1=den_r[:].to_broadcast([P, HID]),
                            op=OP.mult,
                        )
                        nc.vector.tensor_tensor(
                            out=o_f[:], in0=o_f[:], in1=b2_sb[:], op=OP.add
                        )
                        rows = min(SH - t * P, P)
                        nc.sync.dma_start(OUT[t * P : t * P + rows, :], o_f[:rows, :])

        if BUILD_STAGE != "A":
            agg_layer(1)

        if BUILD_STAGE in ("B0", "B1", "B2"):
            stgx = sb.tile([P, HID], fp32, tag="dumm")
            for t in range(cfg.n_tiles):
                rows = min(SH - t * P, P)
                nc.vector.tensor_copy(stgx[:rows, :], b2_sb[:rows, :])
                nc.sync.dma_start(OUT[t * P : t * P + rows, :], stgx[:rows, :])

        if BUILD_STAGE in ("C", "D"):
            if cfg.n_cores > 1:
                nc.gpsimd.collective_compute(
                    "AllGather",
                    OP.bypass,
                    replica_groups=[list(range(cfg.n_cores))],
                    ins=[T2sh.opt()],
                    outs=[T2full.opt()],
                )
            else:
                nc.sync.dma_start(T2full[:, :], T2sh[:, :])

        if BUILD_STAGE == "D":
            agg_layer(2)
        else:
            stg0 = sb.tile([P, HID], fp32, tag="dumm")
            for t in range(cfg.n_tiles):
                rows = min(SH - t * P, P)
                nc.vector.tensor_copy(stg0[:rows, :], b2_sb[:rows, :])
                nc.sync.dma_start(OUT[t * P : t * P + rows, :], stg0[:rows, :])

    nc.compile()
    return nc


# -------------------------------------------------------------------- driver


def make_in_maps(inputs: dict, cfg: Cfg, plan: Plan):
    x = np.asarray(inputs["x"], np.float32)
    W1p, W2p, b1rep, b2rep = prep_weights(inputs, cfg)
    x_pad = np.zeros((cfg.n_pad, cfg.f_in), np.float32)
    x_pad[: cfg.n_nodes] = x
    xT = np.ascontiguousarray(x_pad.T).astype(BF16)
    ident = np.eye(P, dtype=BF16)
    in_maps = []
    for c in range(cfg.n_cores):
        xo = np.zeros((cfg.shard_pad, cfg.f_in), np.float32)
        xo[: cfg.shard] = x[c * cfg.shard : (c + 1) * cfg.shard]
        d = plan.data[c]
        in_maps.append(
            {
                "xT": xT,
                "xTown": np.ascontiguousarray(xo.T).astype(BF16),
                "W1p": W1p,
                "W2p": W2p,
                "b1rep": b1rep,
                "b2rep": b2rep,
                "identity": ident,
                "IDXA": d["IDXA"],
                "IDXD": d["IDXD"],
                "S01": d["S01"],
            }
        )
    return in_maps


def kernel(**inputs) -> np.ndarray:
    cfg = Cfg()
    edge_index = np.asarray(inputs["edge_index"])
    plan = preprocess(edge_index, cfg)
    in_maps = make_in_maps(inputs, cfg, plan)
    nc = build_kernel(cfg, plan)

    from concourse.bass_utils import run_bass_kernel_spmd

    res = run_bass_kernel_spmd(nc, in_maps, core_ids=list(range(cfg.n_cores)))
    out = np.concatenate([r["out"] for r in res.results], axis=0)
    return np.ascontiguousarray(out).astype(np.float32)

